# revision 1
# baseline (speedup 1.0000x reference)
"""Causal self-attention (B=4, T=2048, D=1024, H=16) on 8 Trainium2 NeuronCores.

Sharding: data-parallel over batch (4) x tensor-parallel over heads (2 groups
of 8 heads) = 8 cores. Each core computes q/k/v projections for its 8 heads,
head-local attention, and a partial out-projection; the host sums the two
partials per batch element (the out_proj all-reduce).

On-core layout ("dim-major" activations):
  qT, kT  [128, 4, 2048]  - per head-pair g: partition = channel (2 heads x 64),
                            free = token. Produced by W.T @ x.T matmuls.
  vA      [128, 8, 16, 65]- token-major V per (head, token-tile) with a ones
                            column (65th) so P@V also yields softmax denominators.
  S^T     [128, 512] PSUM - keys on partitions, queries on free dim; causal
                            tiles restrict the matmul to the valid column range,
                            a [128,128] triangular additive mask handles the
                            diagonal band, and the key-padding mask is the
                            per-partition bias of the exp activation.
  yT      [128, 4, 2048]  - normalized attention output, dim-major: feeds
                            out-proj as stationary operand.
All matmuls run in float32r (1 cycle/row at N>=256, ~1.5e-4 rel err).
"""
import numpy as np

import concourse.bass as bass
import concourse.bacc as bacc
import concourse.tile as tile
from concourse import mybir
from concourse.bass_utils import run_bass_kernel_spmd

F32 = mybir.dt.float32
F32R = mybir.dt.float32r
EXP = mybir.ActivationFunctionType.Exp
MULT = mybir.AluOpType.mult
ADD = mybir.AluOpType.add

B, T, D = 4, 2048, 1024
H = 16            # total heads
HD = 64           # head dim
HC = 8            # heads per core
NP = 4            # head pairs per core
NEG = -1.0e6      # additive mask value (exp -> 0 after *0.125)

_CACHE = {}


def _build():
    nc = bacc.Bacc("TRN2", target_bir_lowering=False, debug=False,
                   enable_asserts=False)
    dt_in = {}

    def din(name, shape):
        dt_in[name] = nc.dram_tensor(name, shape, F32, kind="ExternalInput").ap()
        return dt_in[name]

    xt = din("xt", [D, T])            # x[b].T
    wq = din("wq", [D, 512])          # Wq_slice.T
    wk = din("wk", [D, 512])
    wv = din("wv", [D, 512])
    wo = din("wo", [512, D])          # Wo[:, cols].T
    bqt = din("bqt", [128, NP])
    bkt = din("bkt", [128, NP])
    bv = din("bv", [512])
    bo = din("bo", [D])
    tri = din("tri", [128, 128])      # causal 0/1 multiplicative band mask
    keyb = din("keyb", [128, 16])     # key-padding additive bias per k-tile
    qmt = din("qmt", [128, 16])       # query mask, token-major
    out = nc.dram_tensor("out", [T, D], F32, kind="ExternalOutput").ap()

    def bcast128(ap):
        return bass.AP(tensor=ap.tensor, offset=ap.offset, ap=[[0, 128]] + ap.ap)

    with tile.TileContext(nc) as tc:
        cpool = tc.alloc_tile_pool(name="const", bufs=1)
        tri_t = cpool.tile([128, 128], F32)
        keyb_t = cpool.tile([128, 16], F32)
        qmt_t = cpool.tile([128, 16], F32)
        bqt_t = cpool.tile([128, NP], F32)
        bkt_t = cpool.tile([128, NP], F32)
        bvrep = cpool.tile([128, 512], F32)
        borep = cpool.tile([128, D], F32)
        vone_f = cpool.tile([128, HC, 16], F32)
        nc.sync.dma_start(out=tri_t, in_=tri)
        nc.sync.dma_start(out=keyb_t, in_=keyb)
        nc.sync.dma_start(out=qmt_t, in_=qmt)
        nc.sync.dma_start(out=bqt_t, in_=bqt)
        nc.sync.dma_start(out=bkt_t, in_=bkt)
        nc.vector.memset(vone_f, 1.0)

        wopool = tc.alloc_tile_pool(name="wop", bufs=1)
        wo_t = wopool.tile([128, 4, D], F32R)

        qkvpool = tc.alloc_tile_pool(name="qkv", bufs=1)
        qT = qkvpool.tile([128, NP, T], F32R, tag="qT")
        kT = qkvpool.tile([128, NP, T], F32R, tag="kT")
        vA = qkvpool.tile([128, HC, 16, HD + 1], F32R, tag="vA")
        nc.vector.tensor_copy(vA[:, :, :, HD], vone_f)

        # ---------------- Phase 1: QKV projections ----------------
        with (
            tc.tile_pool(name="wts", bufs=1) as wpool,
            tc.tile_pool(name="xts", bufs=2) as xpool,
            tc.tile_pool(name="ps1", bufs=6, space="PSUM") as ps1,
            tc.tile_pool(name="ps1v", bufs=2, space="PSUM") as ps1v,
        ):
            wq_t = wpool.tile([128, 8, 512], F32R, tag="w")
            wk_t = wpool.tile([128, 8, 512], F32R, tag="wk")
            wv_t = wpool.tile([128, 8, 512], F32R, tag="wv")

            def load_w(wdram, wt, split=False):
                for kd in range(8):
                    eng = nc.sync if (split and kd % 2) else nc.gpsimd
                    eng.dma_start(out=wt[:, kd, :],
                                  in_=wdram[kd * 128:(kd + 1) * 128, :].bitcast(F32R))

            SLICES = [(0, 256), (256, 256), (512, 256), (768, 256), (1024, 512), (1536, 512)]

            def load_x(si):
                t0, tl = SLICES[si]
                tiles = []
                for kd in range(8):
                    xtile = xpool.tile([128, 512], F32R, tag=f"x{kd}", name=f"x{si}_{kd}",
                                       padded_shape=[128, 512])
                    # split the critical first slice across the sync and the
                    # otherwise-idle scalar-engine DMA queues
                    eng = nc.scalar if (si == 0 and kd % 2) else nc.sync
                    eng.dma_start(out=xtile[:, 0:tl],
                                  in_=xt[kd * 128:(kd + 1) * 128, t0:t0 + tl].bitcast(F32R))
                    tiles.append(xtile)
                return tiles

            xts_next = load_x(0)
            load_w(wq, wq_t)
            load_w(wk, wk_t)
            nc.sync.dma_start(out=bvrep, in_=bcast128(bv))
            load_w(wv, wv_t)
            nc.gpsimd.dma_start(out=wo_t, in_=wo.rearrange("(g p) n -> p g n", p=128).bitcast(F32R))
            nc.sync.dma_start(out=borep, in_=bcast128(bo))

            for si in range(len(SLICES)):
                t0, tl = SLICES[si]
                xts = xts_next
                if si + 1 < len(SLICES):
                    xts_next = load_x(si + 1)
                for wt, bias_t, outT in ((wq_t, bqt_t, qT), (wk_t, bkt_t, kT)):
                    for g in range(NP):
                        ps = ps1.tile([128, 512], F32, tag="ps1")
                        for kd in range(8):
                            nc.tensor.matmul(ps[:, 0:tl], wt[:, kd, g * 128:(g + 1) * 128],
                                             xts[kd][:, 0:tl],
                                             start=(kd == 0), stop=(kd == 7))
                        nc.vector.tensor_scalar_add(outT[:, g, t0:t0 + tl], ps[:, 0:tl],
                                                    bias_t[:, g:g + 1])
                for tt in range(tl // 128):
                    gtt = (t0 // 128) + tt
                    ps = ps1v.tile([128, 512], F32, tag="psv", name=f"psv{gtt}")
                    for kd in range(8):
                        nc.tensor.matmul(ps, xts[kd][:, tt * 128:(tt + 1) * 128], wv_t[:, kd, :],
                                         start=(kd == 0), stop=(kd == 7))
                    nc.vector.tensor_add(vA[:, :, gtt, 0:HD],
                                         ps.rearrange("p (h d) -> p h d", h=HC),
                                         bvrep.rearrange("p (h d) -> p h d", h=HC))

        # ---------------- Phase 2: attention ----------------
        ypool = tc.alloc_tile_pool(name="yT", bufs=1)
        yT = ypool.tile([128, NP, T], F32R)
        def dim0bc(ap, n):
            # insert a stride-0 dim after the partition dim (broadcast)
            return bass.AP(tensor=ap.tensor, offset=ap.offset,
                           ap=[ap.ap[0], [0, n]] + ap.ap[1:])

        with (
            tc.tile_pool(name="ptile", bufs=8) as ppool,
            tc.tile_pool(name="dro", bufs=2) as dpool,
            tc.tile_pool(name="rec", bufs=2) as rpool,
            tc.tile_pool(name="pss", bufs=2, space="PSUM") as pss,
            tc.tile_pool(name="psy", bufs=2, space="PSUM") as psy,
        ):
            prev_tail = None
            for pr in range(NP):
                for qt in range(4):
                    q0 = qt * 512
                    nk = qt * 4 + 4
                    # two-bank tiles: head A in cols [0,512), head B in [512,1024)
                    ys2 = psy.tile([HD + 1, 1024], F32, tag="y", name=f"y{pr}_{qt}")
                    for kt in range(nk):
                        c = kt * 128 - q0
                        mm_lo = 0 if c < 0 else min(c, 256)
                        lo = max(c, 0)
                        s2 = pss.tile([128, 1024], F32, tag="s", name=f"s{pr}_{qt}_{kt}")
                        with tc.high_priority(offset=48):
                            for hh in range(2):
                                half = slice(hh * 64, hh * 64 + 64)
                                nc.tensor.matmul(s2[:, hh * 512 + mm_lo:hh * 512 + 512],
                                                 kT[half, pr, kt * 128:(kt + 1) * 128],
                                                 qT[half, pr, q0 + mm_lo:q0 + 512],
                                                 start=True, stop=True)
                        s2v = s2.rearrange("p (b n) -> p b n", b=2)
                        p2 = ppool.tile([128, 1024], F32R, tag="p", name=f"p{pr}_{qt}_{kt}")
                        p2v = p2.rearrange("p (b n) -> p b n", b=2)
                        with tc.high_priority(offset=96):
                            nc.scalar.activation(p2v[:, :, lo:512],
                                                 s2v[:, :, lo:512], EXP,
                                                 bias=keyb_t[:, kt:kt + 1], scale=0.125)
                        if c >= 0:
                            nc.vector.tensor_mul(p2v[:, :, lo:lo + 128],
                                                 p2v[:, :, lo:lo + 128],
                                                 dim0bc(tri_t, 2))
                        for hh in range(2):
                            nc.tensor.matmul(ys2[:, hh * 512 + lo:hh * 512 + 512],
                                             vA[:, 2 * pr + hh, kt, :],
                                             p2[:, hh * 512 + lo:hh * 512 + 512],
                                             start=(kt == 0), stop=(kt == nk - 1),
                                             skip_group_check=True)
                    def tail(pr=pr, qt=qt, q0=q0, ys2=ys2):
                        drow = dpool.tile([1, 1024], F32, tag="dc", name=f"dc{pr}_{qt}")
                        nc.vector.tensor_copy(drow, ys2[HD:HD + 1, :])
                        rec1 = dpool.tile([1, 1024], F32, tag="d", name=f"d{pr}_{qt}")
                        nc.vector.reciprocal_approx_fast(rec1, drow)
                        rec2 = rpool.tile([HD, 1024], F32, tag="r", name=f"r{pr}_{qt}")
                        nc.gpsimd.partition_broadcast(rec2, rec1)
                        for hh in range(2):
                            nc.vector.tensor_mul(yT[hh * 64:hh * 64 + 64, pr, q0:q0 + 512],
                                                 ys2[0:HD, hh * 512:hh * 512 + 512],
                                                 rec2[:, hh * 512:hh * 512 + 512])
                    # defer this iteration's denominator/normalize tail until after
                    # the next iteration's matmul loop so its DVE work doesn't
                    # delay the next wave of exps
                    if prev_tail is not None:
                        prev_tail()
                    prev_tail = tail
            prev_tail()

        # ---------------- Phase 3: out projection ----------------
        with (
            tc.tile_pool(name="ob", bufs=4) as opool,
            tc.tile_pool(name="pso", bufs=4, space="PSUM") as psop,
        ):
            for tt in range(16):
                for oh in range(2):
                    po = psop.tile([128, 512], F32, tag="o", name=f"po{tt}_{oh}")
                    for g in range(NP):
                        nc.tensor.matmul(po, yT[:, g, tt * 128:(tt + 1) * 128],
                                         wo_t[:, g, oh * 512:(oh + 1) * 512],
                                         start=(g == 0), stop=(g == NP - 1))
                    ob = opool.tile([128, 512], F32, tag="ob", name=f"ob{tt}_{oh}")
                    nc.vector.scalar_tensor_tensor(ob, po, qmt_t[:, tt:tt + 1],
                                                   borep[:, oh * 512:(oh + 1) * 512],
                                                   op0=MULT, op1=ADD)
                    nc.sync.dma_start(out=out[tt * 128:(tt + 1) * 128, oh * 512:(oh + 1) * 512],
                                      in_=ob)
        for p in (ypool, qkvpool, wopool, cpool):
            p.release()
    nc.compile()
    return nc


def _host_inputs(x, attention_mask, Wqkv, bqkv, Wo, bo):
    """Build the 8 per-core input maps."""
    x = np.asarray(x, dtype=np.float32)
    attention_mask = np.asarray(attention_mask)
    Wqkv = np.asarray(Wqkv, dtype=np.float32)
    bqkv = np.asarray(bqkv, dtype=np.float32)
    Wo = np.asarray(Wo, dtype=np.float32)
    bo = np.asarray(bo, dtype=np.float32)

    tri = (np.arange(128)[:, None] <= np.arange(128)[None, :]).astype(np.float32)
    zeros_bo = np.zeros_like(bo)
    in_maps = []
    for core in range(8):
        b = core // 2
        hg = core % 2
        cs = hg * 512
        m = attention_mask[b].astype(bool)
        keyb = np.where(m, 0.0, NEG).astype(np.float32).reshape(16, 128).T.copy()
        qmt = m.astype(np.float32).reshape(16, 128).T.copy()
        in_maps.append({
            "xt": np.ascontiguousarray(x[b].T),
            "wq": np.ascontiguousarray(Wqkv[cs:cs + 512, :].T),
            "wk": np.ascontiguousarray(Wqkv[D + cs:D + cs + 512, :].T),
            "wv": np.ascontiguousarray(Wqkv[2 * D + cs:2 * D + cs + 512, :].T),
            "wo": np.ascontiguousarray(Wo[:, cs:cs + 512].T),
            "bqt": np.ascontiguousarray(bqkv[cs:cs + 512].reshape(NP, 128).T),
            "bkt": np.ascontiguousarray(bqkv[D + cs:D + cs + 512].reshape(NP, 128).T),
            "bv": bqkv[2 * D + cs:2 * D + cs + 512].copy(),
            "bo": bo if hg == 0 else zeros_bo,
            "tri": tri,
            "keyb": keyb,
            "qmt": qmt,
        })
    return in_maps


def kernel(x, attention_mask, Wqkv, bqkv, Wo, bo, _trace=False, _trace_kwargs=None):
    if "nc" not in _CACHE:
        _CACHE["nc"] = _build()
    nc = _CACHE["nc"]
    in_maps = _host_inputs(x, attention_mask, Wqkv, bqkv, Wo, bo)
    kwargs = {}
    if _trace:
        kwargs["trace"] = True
        if _trace_kwargs:
            kwargs.update(_trace_kwargs)
    res = run_bass_kernel_spmd(nc, in_maps, core_ids=list(range(8)), **kwargs)
    _CACHE["last_result"] = res
    out = np.empty((B, T, D), dtype=np.float32)
    for b in range(B):
        out[b] = res.results[2 * b]["out"] + res.results[2 * b + 1]["out"]
    return out



# revision 3
# speedup vs baseline: 1.0807x; 1.0807x over previous
"""Causal self-attention (B=4, T=2048, D=1024, H=16) on 8 Trainium2 NeuronCores.

Sharding: data-parallel over batch (4) x tensor-parallel over heads (2 groups
of 8 heads) = 8 cores. Each core computes q/k/v projections for its 8 heads,
head-local attention, and a partial out-projection; the host sums the two
partials per batch element (the out_proj all-reduce).

Dtype strategy (validated offline to rel err ~1.2e-3 vs the f32 oracle):
  - QKV projections run as 3 fp8e4m3 DoubleRow passes accumulating in f32
    PSUM: x8@W8 + xres@W8 + x8@Wres, where *res are the fp8 quantization
    residuals (representable directly thanks to e4m3 subnormals). DoubleRow
    contracts 256 rows per matmul at 0.5 cycles/column, so the 3 passes cost
    6N cycles per 128 output channels vs 8N for fp32r.
  - Attention (S = K^T Q, exp, P@V) runs in fp16: 1 cycle/row at any width,
    which also allows exact causal column ranges on the diagonal tiles.
  - Out-projection runs in fp16.
Scales (sx for x, per-tensor sw for each W slice) are data-driven on the host
and shipped as per-partition scalars, so all 8 cores run one SPMD program.

On-core layout ("dim-major" activations):
  qT, kT  [128, 4, 2048] fp16 - per head-pair g: partition = channel
                            (2 heads x 64), free = token.
  vA      [128, 8, 16, 65] fp16 - token-major V per (head, token-tile) with a
                            ones column (65th) so P@V also yields softmax
                            denominators.
  S^T     [128, 512] PSUM - keys on partitions, queries on free dim; causal
                            tiles restrict the matmul to the exact valid
                            column range, a [128,128] fp16 triangular mask
                            handles the diagonal band, and the key-padding
                            mask is the per-partition bias of the exp.
  yT      [128, 4, 2048] fp16 - normalized attention output, dim-major: feeds
                            out-proj as stationary operand.
"""
import numpy as np
import ml_dtypes

import concourse.bass as bass
import concourse.bacc as bacc
import concourse.tile as tile
from concourse import mybir
from concourse.bass_utils import run_bass_kernel_spmd

F32 = mybir.dt.float32
F16 = mybir.dt.float16
FP8 = mybir.dt.float8e4
E4M3 = ml_dtypes.float8_e4m3
DR = mybir.MatmulPerfMode.DoubleRow
EXP = mybir.ActivationFunctionType.Exp
MULT = mybir.AluOpType.mult
ADD = mybir.AluOpType.add

B, T, D = 4, 2048, 1024
H = 16            # total heads
HD = 64           # head dim
HC = 8            # heads per core
NP = 4            # head pairs per core
NEG = -1.0e6      # additive mask value (exp -> 0 after *0.125)
SX = 8.0          # fp8 scale for x

_CACHE = {}


def _build():
    nc = bacc.Bacc("TRN2", target_bir_lowering=False, debug=False,
                   enable_asserts=False)
    dt_in = {}

    def din(name, shape, dt=F32):
        dt_in[name] = nc.dram_tensor(name, shape, dt, kind="ExternalInput").ap()
        return dt_in[name]

    x8 = din("x8", [128, 4, 2, T], FP8)    # fp8(x[b].T * SX), chan = u*256+j*128+p
    xr = din("xr", [128, 4, 2, T], FP8)    # residual fp8(x*SX - x8)
    w8q = din("w8q", [128, 2, 4, 2, 512], FP8)  # [p, (W8|Wres), u, j, outchan]
    w8k = din("w8k", [128, 2, 4, 2, 512], FP8)
    w8v = din("w8v", [128, 2, 4, 2, 512], FP8)
    w16o = din("w16o", [512, D], F16)      # Wo[:, cols].T
    bqt = din("bqt", [128, NP])
    bkt = din("bkt", [128, NP])
    bv = din("bv", [512])
    bo = din("bo", [D])
    dsc = din("dsc", [128, 3])             # descale columns: 1/(SX*swq) etc.
    tri = din("tri", [128, 128], F16)      # causal 0/1 multiplicative band mask
    keyb = din("keyb", [128, 16])          # key-padding additive bias per k-tile
    qmt = din("qmt", [128, 16])            # query mask, token-major
    out = nc.dram_tensor("out", [T, D], F32, kind="ExternalOutput").ap()

    def bcast128(ap):
        return bass.AP(tensor=ap.tensor, offset=ap.offset, ap=[[0, 128]] + ap.ap)

    def fbc(ap, n):
        # broadcast a [128, 1] scalar across n free columns (stride-0 free dim)
        return bass.AP(tensor=ap.tensor, offset=ap.offset, ap=[ap.ap[0], [0, n]])

    with tile.TileContext(nc) as tc:
        cpool = tc.alloc_tile_pool(name="const", bufs=1)
        tri_t = cpool.tile([128, 128], F16)
        keyb_t = cpool.tile([128, 16], F32)
        qmt_t = cpool.tile([128, 16], F32)
        bqt_t = cpool.tile([128, NP], F32)
        bkt_t = cpool.tile([128, NP], F32)
        dsc_t = cpool.tile([128, 3], F32)
        bvrep = cpool.tile([128, 512], F32)
        borep = cpool.tile([128, D], F32)
        nc.sync.dma_start(out=tri_t, in_=tri)
        nc.sync.dma_start(out=keyb_t, in_=keyb)
        nc.sync.dma_start(out=qmt_t, in_=qmt)
        nc.sync.dma_start(out=bqt_t, in_=bqt)
        nc.sync.dma_start(out=bkt_t, in_=bkt)
        nc.sync.dma_start(out=dsc_t, in_=dsc)

        wopool = tc.alloc_tile_pool(name="wop", bufs=1)
        wo_t = wopool.tile([128, 4, D], F16)

        qkvpool = tc.alloc_tile_pool(name="qkv", bufs=1)
        qT = qkvpool.tile([128, NP, T], F16, tag="qT")
        kT = qkvpool.tile([128, NP, T], F16, tag="kT")
        vA = qkvpool.tile([128, HC, 16, HD + 1], F16, tag="vA")
        nc.vector.memset(vA[:, :, :, HD], 1.0)

        # ---------------- Phase 1: QKV projections ----------------
        with (
            tc.tile_pool(name="wts", bufs=1) as wpool,
            tc.tile_pool(name="xts", bufs=2) as xpool,
            tc.tile_pool(name="ps1", bufs=6, space="PSUM") as ps1,
            tc.tile_pool(name="ps1v", bufs=2, space="PSUM") as ps1v,
        ):
            wq_t = wpool.tile([128, 2, 4, 2, 512], FP8, tag="w")
            wk_t = wpool.tile([128, 2, 4, 2, 512], FP8, tag="wk")
            wv_t = wpool.tile([128, 2, 4, 2, 512], FP8, tag="wv")

            def load_w(wdram, wt, wv, eng):
                # one DMA per (tensor, W8|Wres): 4KB/partition
                eng.dma_start(out=wt[:, wv], in_=wdram[:, wv])

            SLICES = [(0, 256), (256, 256), (512, 256), (768, 256), (1024, 512), (1536, 512)]

            def load_x(si):
                t0, tl = SLICES[si]
                tiles = []
                for u in range(4):
                    for nm, src in (("a", x8), ("r", xr)):
                        xtile = xpool.tile([128, 2, 512], FP8, tag=f"x{nm}{u}",
                                           name=f"x{nm}{si}_{u}",
                                           padded_shape=[128, 2, 512])
                        eng = nc.scalar if (si == 0 and u % 2) else nc.sync
                        eng.dma_start(out=xtile[:, :, 0:tl],
                                      in_=src[:, u, :, t0:t0 + tl])
                        tiles.append(xtile)
                return tiles  # [a0, r0, a1, r1, ...]

            xts_next = load_x(0)
            # pass-A weights first (they gate the first matmuls)
            load_w(w8q, wq_t, 0, nc.sync)
            load_w(w8k, wk_t, 0, nc.scalar)
            load_w(w8v, wv_t, 0, nc.gpsimd)
            nc.sync.dma_start(out=bvrep, in_=bcast128(bv))
            load_w(w8q, wq_t, 1, nc.scalar)
            load_w(w8k, wk_t, 1, nc.sync)
            load_w(w8v, wv_t, 1, nc.scalar)
            nc.gpsimd.dma_start(out=wo_t, in_=w16o.rearrange("(g p) n -> p g n", p=128))
            nc.sync.dma_start(out=borep, in_=bcast128(bo))

            for si in range(len(SLICES)):
                t0, tl = SLICES[si]
                xts = xts_next
                if si + 1 < len(SLICES):
                    xts_next = load_x(si + 1)
                for wt, bias_t, dcol, outT in ((wq_t, bqt_t, 0, qT), (wk_t, bkt_t, 1, kT)):
                    for g in range(NP):
                        ps = ps1.tile([128, 512], F32, tag="ps1")
                        mm = 0
                        # (x-operand, W-variant) passes: A=(x8,W8) B=(xr,W8) C=(x8,Wres)
                        for xoff, wv in ((0, 0), (1, 0), (0, 1)):
                            for u in range(4):
                                nc.tensor.matmul(ps[:, 0:tl],
                                                 wt[:, wv, u, :, g * 128:(g + 1) * 128],
                                                 xts[2 * u + xoff][:, :, 0:tl],
                                                 start=(mm == 0), stop=(mm == 11),
                                                 perf_mode=DR)
                                mm += 1
                        nc.vector.scalar_tensor_tensor(
                            outT[:, g, t0:t0 + tl], ps[:, 0:tl],
                            dsc_t[:, dcol:dcol + 1], fbc(bias_t[:, g:g + 1], tl),
                            op0=MULT, op1=ADD)
                for tt in range(tl // 128):
                    gtt = (t0 // 128) + tt
                    ps = ps1v.tile([128, 512], F32, tag="psv", name=f"psv{gtt}")
                    mm = 0
                    for xoff, wv in ((0, 0), (1, 0), (0, 1)):
                        for u in range(4):
                            nc.tensor.matmul(ps,
                                             xts[2 * u + xoff][:, :, tt * 128:(tt + 1) * 128],
                                             wv_t[:, wv, u],
                                             start=(mm == 0), stop=(mm == 11),
                                             perf_mode=DR)
                            mm += 1
                    nc.vector.scalar_tensor_tensor(
                        vA[:, :, gtt, 0:HD],
                        ps.rearrange("p (h d) -> p h d", h=HC),
                        dsc_t[:, 2:3],
                        bvrep.rearrange("p (h d) -> p h d", h=HC),
                        op0=MULT, op1=ADD)

        # ---------------- Phase 2: attention ----------------
        ypool = tc.alloc_tile_pool(name="yT", bufs=1)
        yT = ypool.tile([128, NP, T], F16)

        def dim0bc(ap, n):
            # insert a stride-0 dim after the partition dim (broadcast)
            return bass.AP(tensor=ap.tensor, offset=ap.offset,
                           ap=[ap.ap[0], [0, n]] + ap.ap[1:])

        with (
            tc.tile_pool(name="ptile", bufs=8) as ppool,
            tc.tile_pool(name="dro", bufs=2) as dpool,
            tc.tile_pool(name="rec", bufs=2) as rpool,
            tc.tile_pool(name="pss", bufs=2, space="PSUM") as pss,
            tc.tile_pool(name="psy", bufs=2, space="PSUM") as psy,
        ):
            prev_tail = None
            for pr in range(NP):
                for qt in range(4):
                    q0 = qt * 512
                    nk = qt * 4 + 4
                    # two-bank tiles: head A in cols [0,512), head B in [512,1024)
                    ys2 = psy.tile([HD + 1, 1024], F32, tag="y", name=f"y{pr}_{qt}")
                    for kt in range(nk):
                        c = kt * 128 - q0
                        lo = max(c, 0)
                        s2 = pss.tile([128, 1024], F32, tag="s", name=f"s{pr}_{qt}_{kt}")
                        with tc.high_priority(offset=48):
                            for hh in range(2):
                                half = slice(hh * 64, hh * 64 + 64)
                                nc.tensor.matmul(s2[:, hh * 512 + lo:hh * 512 + 512],
                                                 kT[half, pr, kt * 128:(kt + 1) * 128],
                                                 qT[half, pr, q0 + lo:q0 + 512],
                                                 start=True, stop=True)
                        s2v = s2.rearrange("p (b n) -> p b n", b=2)
                        p2 = ppool.tile([128, 1024], F16, tag="p", name=f"p{pr}_{qt}_{kt}")
                        p2v = p2.rearrange("p (b n) -> p b n", b=2)
                        with tc.high_priority(offset=96):
                            nc.scalar.activation(p2v[:, :, lo:512],
                                                 s2v[:, :, lo:512], EXP,
                                                 bias=keyb_t[:, kt:kt + 1], scale=0.125)
                        if c >= 0:
                            nc.vector.tensor_mul(p2v[:, :, lo:lo + 128],
                                                 p2v[:, :, lo:lo + 128],
                                                 dim0bc(tri_t, 2))
                        for hh in range(2):
                            nc.tensor.matmul(ys2[:, hh * 512 + lo:hh * 512 + 512],
                                             vA[:, 2 * pr + hh, kt, :],
                                             p2[:, hh * 512 + lo:hh * 512 + 512],
                                             start=(kt == 0), stop=(kt == nk - 1),
                                             skip_group_check=True)
                    def tail(pr=pr, qt=qt, q0=q0, ys2=ys2):
                        drow = dpool.tile([1, 1024], F32, tag="dc", name=f"dc{pr}_{qt}")
                        nc.vector.tensor_copy(drow, ys2[HD:HD + 1, :])
                        rec1 = dpool.tile([1, 1024], F32, tag="d", name=f"d{pr}_{qt}")
                        nc.vector.reciprocal_approx_fast(rec1, drow)
                        rec2 = rpool.tile([HD, 1024], F32, tag="r", name=f"r{pr}_{qt}")
                        nc.gpsimd.partition_broadcast(rec2, rec1)
                        for hh in range(2):
                            nc.vector.tensor_mul(yT[hh * 64:hh * 64 + 64, pr, q0:q0 + 512],
                                                 ys2[0:HD, hh * 512:hh * 512 + 512],
                                                 rec2[:, hh * 512:hh * 512 + 512])
                    # defer this iteration's denominator/normalize tail until after
                    # the next iteration's matmul loop so its DVE work doesn't
                    # delay the next wave of exps
                    if prev_tail is not None:
                        prev_tail()
                    prev_tail = tail
            prev_tail()

        # ---------------- Phase 3: out projection ----------------
        with (
            tc.tile_pool(name="ob", bufs=4) as opool,
            tc.tile_pool(name="pso", bufs=4, space="PSUM") as psop,
        ):
            for tt in range(16):
                for oh in range(2):
                    po = psop.tile([128, 512], F32, tag="o", name=f"po{tt}_{oh}")
                    for g in range(NP):
                        nc.tensor.matmul(po, yT[:, g, tt * 128:(tt + 1) * 128],
                                         wo_t[:, g, oh * 512:(oh + 1) * 512],
                                         start=(g == 0), stop=(g == NP - 1))
                    ob = opool.tile([128, 512], F32, tag="ob", name=f"ob{tt}_{oh}")
                    nc.vector.scalar_tensor_tensor(ob, po, qmt_t[:, tt:tt + 1],
                                                   borep[:, oh * 512:(oh + 1) * 512],
                                                   op0=MULT, op1=ADD)
                    nc.sync.dma_start(out=out[tt * 128:(tt + 1) * 128, oh * 512:(oh + 1) * 512],
                                      in_=ob)
        for p in (ypool, qkvpool, wopool, cpool):
            p.release()
    nc.compile()
    return nc


def _q8(a):
    y = np.asarray(a).astype(E4M3)
    return y


def _host_inputs(x, attention_mask, Wqkv, bqkv, Wo, bo):
    """Build the 8 per-core input maps."""
    x = np.asarray(x, dtype=np.float32)
    attention_mask = np.asarray(attention_mask)
    Wqkv = np.asarray(Wqkv, dtype=np.float32)
    bqkv = np.asarray(bqkv, dtype=np.float32)
    Wo = np.asarray(Wo, dtype=np.float32)
    bo = np.asarray(bo, dtype=np.float32)

    tri = (np.arange(128)[:, None] <= np.arange(128)[None, :]).astype(np.float16)
    zeros_bo = np.zeros_like(bo)
    in_maps = []
    for core in range(8):
        b = core // 2
        hg = core % 2
        cs = hg * 512
        m = attention_mask[b].astype(bool)
        keyb = np.where(m, 0.0, NEG).astype(np.float32).reshape(16, 128).T.copy()
        qmt = m.astype(np.float32).reshape(16, 128).T.copy()

        xt = x[b].T * SX                       # [1024, 2048]
        x8f = _q8(xt)
        xrf = _q8(xt - x8f.astype(np.float32))
        x8h = np.ascontiguousarray(
            x8f.reshape(4, 2, 128, T).transpose(2, 0, 1, 3))
        xrh = np.ascontiguousarray(
            xrf.reshape(4, 2, 128, T).transpose(2, 0, 1, 3))

        dscs = []
        wts = {}
        for nm, W in (("w8q", Wqkv[cs:cs + 512]),
                      ("w8k", Wqkv[D + cs:D + cs + 512]),
                      ("w8v", Wqkv[2 * D + cs:2 * D + cs + 512])):
            sw = 64.0 / max(np.abs(W).max(), 1e-9)
            Wt = W.T * sw                      # [1024, 512]
            W8 = _q8(Wt)
            Wres = _q8(Wt - W8.astype(np.float32))
            arr = np.stack([W8, Wres], 0)      # [2, 1024, 512]
            wts[nm] = np.ascontiguousarray(
                arr.reshape(2, 4, 2, 128, 512).transpose(3, 0, 1, 2, 4))
            dscs.append(1.0 / (SX * sw))
        dsc = np.tile(np.array(dscs, np.float32)[None, :], (128, 1))

        in_maps.append({
            "x8": x8h,
            "xr": xrh,
            "w8q": wts["w8q"],
            "w8k": wts["w8k"],
            "w8v": wts["w8v"],
            "w16o": np.ascontiguousarray(Wo[:, cs:cs + 512].T).astype(np.float16),
            "bqt": np.ascontiguousarray(bqkv[cs:cs + 512].reshape(NP, 128).T),
            "bkt": np.ascontiguousarray(bqkv[D + cs:D + cs + 512].reshape(NP, 128).T),
            "bv": bqkv[2 * D + cs:2 * D + cs + 512].copy(),
            "bo": bo if hg == 0 else zeros_bo,
            "dsc": dsc,
            "tri": tri,
            "keyb": keyb,
            "qmt": qmt,
        })
    return in_maps


def kernel(x, attention_mask, Wqkv, bqkv, Wo, bo, _trace=False, _trace_kwargs=None):
    if "nc" not in _CACHE:
        _CACHE["nc"] = _build()
    nc = _CACHE["nc"]
    in_maps = _host_inputs(x, attention_mask, Wqkv, bqkv, Wo, bo)
    kwargs = {}
    if _trace:
        kwargs["trace"] = True
        if _trace_kwargs:
            kwargs.update(_trace_kwargs)
    res = run_bass_kernel_spmd(nc, in_maps, core_ids=list(range(8)), **kwargs)
    _CACHE["last_result"] = res
    out = np.empty((B, T, D), dtype=np.float32)
    for b in range(B):
        out[b] = res.results[2 * b]["out"] + res.results[2 * b + 1]["out"]
    return out
